# revision 77
# baseline (speedup 1.0000x reference)
"""3-layer GAT on Trainium2, 8-core SPMD Bass kernel (v3).

Design (dst-partitioned, edge-gather, 128-wide blocks):
  - Nodes partitioned contiguously across 8 cores (6250/core); each core owns
    all edges whose dst is local, so segment-softmax/scatter-sum are local.
  - Per layer each core computes z_aug rows [z(D) | 1 | s_fp16] for its nodes
    (s = z.a_src), writes them to a DRAM table, AllGathers the full table
    (split in two halves so the first half overlaps the tail of the layer),
    then per 128-dst block dma_gathers the z_aug rows of that block's edge
    sources, computes E = exp(leaky(s + t[dst])) densely over the 128
    candidate dsts (fp16), masks with a one-hot (seg==iota) matrix, and
    reduces with PE matmuls: U[dst, :] = sum_e ex_e * z_aug[src_e]; the
    table's constant-1 column yields the softmax denominator for free.
  - t = z.a_dst stays core-local ([npc] values, broadcast per block via a
    PE outer product directly into fp16 PSUM).
  - dma_gather descriptor generation runs on the Q7 core pair selected by
    queue_num (cpu_id/2 == queue_num): blocks rotate over the 4 SWDGE queues
    and a deep (RB=9) gather ring keeps all 4 pairs generating concurrently.
  - Layer 3 rows are 512B (elem 256) instead of 768B.
  - h is transposed on the host (fp16) so priming needs no PE transposes.

Host preprocessing touches only src/dst (grouping, padding, int16 packing)
and weight layout; all float graph compute happens on device.
"""

import sys

import numpy as np

sys.path.insert(0, "/opt/trn_rl_repo")

import ml_dtypes  # noqa: E402

# --- problem constants (hardcoded) ---
N_NODES = 50000
N_EDGES = 800000
DIM_IN = 256
DIM_HID = 256
DIM_OUT = 128
N_CORES = 8

BLOCK = 128      # dst nodes per mask block
MAXCK = 8        # max chunks (x128 idx) per dma_gather call (ring capacity)
PA_BLOCKS = 32   # part-A blocks per core (part-major AllGather split)
PA1_BLOCKS = 16  # part-A1 blocks (0..15); A2 = 16..31
PB1_BLOCKS = 12  # part-B1 blocks (32..43); rest is part B2
P = 128

NEG_SLOPE = 0.01


def _cdiv(a, b):
    return -(-a // b)


# ---------------------------------------------------------------------------
# host-side graph plan (pure index preprocessing)
# ---------------------------------------------------------------------------

def _pack_part(ids, lo_deg, hi_deg, n_cores, nblk, sizes_per_core, cap_lo,
               cap_hi):
    """Assign the nodes `ids` to n_cores*nblk bins (bin = (core, block)) so
    each bin's lo/hi in-edge counts stay under the caps and loads balance.
    Returns binm: per-id bin index (core*nblk + blk). Pure preprocessing."""
    lo, hi = lo_deg[ids], hi_deg[ids]
    order = np.argsort(-(lo + hi))
    B = n_cores * nblk
    sizes = np.tile(np.asarray(sizes_per_core), n_cores)
    slo = np.zeros(B)
    shi = np.zeros(B)
    cnt = np.zeros(B, np.int64)
    binm = np.full(len(ids), -1, np.int64)
    for j in order:
        l_, h_ = lo[j], hi[j]
        ov = (np.maximum(slo + l_ - cap_lo, 0)
              + np.maximum(shi + h_ - cap_hi, 0))
        score = ov * 1e6 \
            + np.maximum((slo + l_) * (1024.0 / cap_lo),
                         (shi + h_) * (1024.0 / cap_hi)) \
            + (cnt >= sizes) * 1e18
        bb = int(np.argmin(score))
        binm[j] = bb
        slo[bb] += l_
        shi[bb] += h_
        cnt[bb] += 1
    # swap refinement: fix bins with slo or shi over cap via 1-for-1 node
    # swaps (bin sizes preserved, so partial bins participate too)
    for _ in range(2000):
        over = np.where((slo > cap_lo) | (shi > cap_hi))[0]
        if len(over) == 0:
            break
        ob = int(over[0])
        members = np.where(binm == ob)[0]
        msort = members[np.argsort(-(lo[members] + hi[members]))]
        fixed = False
        for a in msort[:96]:
            la, ha = lo[a], hi[a]
            needl = slo[ob] - cap_lo
            needh = shi[ob] - cap_hi
            okb = ((lo <= la - needl)
                   & (hi <= ha - needh)
                   & (slo[binm] + la - lo <= cap_lo)
                   & (shi[binm] + ha - hi <= cap_hi)
                   & (binm != ob))
            w = np.where(okb)[0]
            if len(w):
                bn_ = int(w[0])
                ub = int(binm[bn_])
                binm[a], binm[bn_] = ub, ob
                slo[ob] += lo[bn_] - la
                shi[ob] += hi[bn_] - ha
                slo[ub] += la - lo[bn_]
                shi[ub] += ha - hi[bn_]
                fixed = True
                break
        if not fixed:
            break
    assert (slo <= cap_lo).all() and (shi <= cap_hi).all(), \
        (slo.max(), shi.max())
    return binm


def _pack_nodes(src, dst, n_nodes, n_cores, npc, pa, nb):
    """Two-stage part-major packing. Part A = blocks [0, pa/128) of every
    core (table rows 0..8*pa-1, int16-addressable); part B = the rest.
    Stage 1 picks the part-A node set (degree-balanced); stage 2 packs each
    part's nodes into its (core, block) bins under per-block gather-call
    caps (lo = part-A sources <= 2048 = 2 calls; hi <= 1024 = 1 call).
    Returns perm: position -> node id."""
    pb = npc - pa
    na = pa * n_cores
    deg = np.bincount(dst, minlength=n_nodes) + 1e-3 * np.bincount(
        src, minlength=n_nodes)
    order = np.argsort(-deg)
    partA = np.zeros(n_nodes, bool)
    ca = cb = 0
    nb_tot = n_nodes - na
    for j in order:
        if ca < na and (cb >= nb_tot or ca * nb_tot <= cb * na):
            partA[j] = True
            ca += 1
        else:
            cb += 1
    assert ca == na and cb == nb_tot
    lo_deg = np.bincount(dst[partA[src]], minlength=n_nodes)
    hi_deg = np.bincount(dst[~partA[src]], minlength=n_nodes)

    nblk_a = pa // BLOCK
    nblk_b = _cdiv(pb, BLOCK)
    lastsz = pb - (nblk_b - 1) * BLOCK
    idsA = np.where(partA)[0]
    idsB = np.where(~partA)[0]
    binA = _pack_part(idsA, lo_deg, hi_deg, n_cores, nblk_a,
                      [BLOCK] * nblk_a, 1664, 896)
    binB = _pack_part(idsB, lo_deg, hi_deg, n_cores, nblk_b,
                      [BLOCK] * (nblk_b - 1) + [lastsz], 1664, 896)

    perm = np.zeros(n_nodes, np.int64)
    for ids, binm, nblk, off in [(idsA, binA, nblk_a, 0),
                                 (idsB, binB, nblk_b, pa)]:
        for bb in range(n_cores * nblk):
            cc_, blk = bb // nblk, bb % nblk
            nodes = ids[np.where(binm == bb)[0]]
            pos0 = cc_ * npc + off + blk * BLOCK
            perm[pos0: pos0 + len(nodes)] = nodes
    return perm


def build_plan(src, dst, n_nodes, n_cores):
    src = np.asarray(src).astype(np.int64)
    dst = np.asarray(dst).astype(np.int64)
    assert n_nodes % n_cores == 0
    npc = n_nodes // n_cores
    pa = PA_BLOCKS * BLOCK                      # part-A rows per core
    pb = npc - pa
    assert pa * n_cores <= 32768 and pb * n_cores <= 32768

    nb = _cdiv(npc, BLOCK)
    perm = _pack_nodes(src, dst, n_nodes, n_cores, npc, pa, nb)
    inv = np.zeros(n_nodes, np.int64)
    inv[perm] = np.arange(n_nodes)
    # rewrite the graph in position space
    src = inv[src]
    dst = inv[dst]

    c = dst // npc
    loc = dst - c * npc
    b = loc // BLOCK
    sg = loc % BLOCK
    # table row of a source position, part-major: part A (r < pa) occupies
    # rows [0, 8*pa) = int16 range; part B rows [8*pa, n_nodes)
    sc = src // npc
    sr = src - sc * npc
    hf = (sr >= pa).astype(np.int64)
    pb1 = PB1_BLOCKS * BLOCK
    pb2 = pb - pb1
    # lo-table rows: four uniform 1024-row sub-parts (A1..A4), part-major;
    # hi-table rows: B1 then B2 (part-major)
    ai = np.minimum(sr // 1024, 3)
    lo_row = ai * (n_cores * 1024) + sc * 1024 + (sr - ai * 1024)
    hi_row = np.where(sr < pa + pb1,
                      sc * pb1 + (sr - pa),
                      n_cores * pb1 + sc * pb2 + (sr - pa - pb1))
    src_row = np.where(hf == 0, lo_row, hi_row)

    key = (c * nb + b) * 2 + hf
    cnt = np.bincount(key, minlength=n_cores * nb * 2).reshape(n_cores, nb, 2)
    nmax = cnt.max(axis=0)  # [nb, 2] max edges per (block, half) across cores
    ck = _cdiv(nmax, 128)   # chunks per (block, half), shared across cores
    dead = ck.sum(axis=1) == 0
    ck[dead, 0] = 1  # keep >=1 chunk per block so U accumulation happens

    order = np.lexsort((src_row, hf, b, c))
    gsize = cnt.reshape(-1)
    gstart = np.zeros_like(gsize)
    gstart[1:] = np.cumsum(gsize)[:-1]

    # per-block layout offsets
    blocks = []
    seg_cols = 0
    i16lo = i16hi = 0
    for bb in range(nb):
        cklo, ckhi = int(ck[bb, 0]), int(ck[bb, 1])
        n16lo = _cdiv(int(nmax[bb, 0]), 16)
        n16hi = _cdiv(int(nmax[bb, 1]), 16)
        blocks.append(dict(
            cklo=cklo, ckhi=ckhi, cktot=cklo + ckhi,
            nlo=int(nmax[bb, 0]), nhi=int(nmax[bb, 1]),
            n16lo=n16lo, n16hi=n16hi,
            sgo=seg_cols, ilo=i16lo, ihi=i16hi,
        ))
        seg_cols += cklo + ckhi
        i16lo += n16lo
        i16hi += n16hi

    def wrap16(a):
        S = len(a) // 16
        w = a.reshape(S, 16).T
        return np.tile(w, (8, 1))

    idxlo = np.full((n_cores, P, i16lo), -1, np.int16)
    idxhi = np.full((n_cores, P, i16hi), -1, np.int16)
    seg = np.full((n_cores, P, seg_cols), 255.0, np.float32)

    for cc_ in range(n_cores):
        for bb in range(nb):
            bl = blocks[bb]
            for h, (ckh, n16, off, itab) in enumerate([
                (bl["cklo"], bl["n16lo"], bl["ilo"], idxlo),
                (bl["ckhi"], bl["n16hi"], bl["ihi"], idxhi),
            ]):
                n = int(cnt[cc_, bb, h])
                if n16 == 0:
                    continue
                s0 = gstart[(cc_ * nb + bb) * 2 + h]
                e = order[s0: s0 + n]
                # pad with idx 0 (gathers a valid row; seg=255 masks it out).
                # Trailing-negative trimming crashes the DGE ring bookkeeping,
                # so padded slots must stay gatherable.
                ids = np.zeros(n16 * 16, np.int64)
                ids[:n] = src_row[e]
                itab[cc_, :, off: off + n16] = wrap16(ids)
                # seg values: slot j -> partition j%128, chunk j//128
                segs = np.full(ckh * 128, 255, np.int64)
                segs[:n] = sg[e]
                cs = bl["sgo"] + (bl["cklo"] if h else 0)
                seg[cc_, :, cs: cs + ckh] = segs.reshape(ckh, 128).T

    meta = dict(
        n_cores=n_cores, n_nodes=n_nodes, npc=npc, pa=pa, pb=pb, nb=nb,
        blocks=blocks, seg_cols=seg_cols, i16lo=i16lo, i16hi=i16hi,
        ckmax=max(bl["cktot"] for bl in blocks), perm=perm,
        n_calls=sum(_cdiv(bl["nlo"], 1024) + _cdiv(bl["nhi"], 1024)
                    for bl in blocks),
        n_desc=sum(bl["nlo"] + bl["nhi"] for bl in blocks),
    )
    per_core = dict(
        idxlo=idxlo, idxhi=idxhi,
        seg=seg.astype(ml_dtypes.bfloat16),
    )
    return meta, per_core


def const_inputs():
    iota = np.tile(np.arange(BLOCK, dtype=np.float32), (P, 1)).astype(
        ml_dtypes.bfloat16
    )
    id16 = np.eye(P, dtype=np.float16)
    id32 = np.eye(P, dtype=np.float32)
    return {"iota": iota, "id16": id16, "id32": id32}


def build_waug(W, A):
    d_out = W.shape[0]
    Wt = W.T.astype(np.float64)
    a_s = A[0, :d_out].astype(np.float64)
    a_d = A[0, d_out:].astype(np.float64)
    waug = np.concatenate([Wt, (Wt @ a_s)[:, None], (Wt @ a_d)[:, None]], axis=1)
    return waug.astype(np.float16)


# ---------------------------------------------------------------------------
# device program
# ---------------------------------------------------------------------------

def build_nc(meta, debug=False):
    import concourse.bacc as bacc
    import concourse.bass as bass
    import concourse.mybir as mybir
    import concourse.tile as tile
    from concourse.library_config import mlp

    dt = mybir.dt
    AP = bass.AP
    Alu = bass.mybir.AluOpType
    Act = bass.mybir.ActivationFunctionType

    npc = meta["npc"]
    pa = meta["pa"]
    pb = meta["pb"]
    N = meta["n_nodes"]
    nb = meta["nb"]
    n_cores = meta["n_cores"]
    blocks = meta["blocks"]
    CKMAX = meta["ckmax"]
    RB = 8   # g ring depth (deep: keeps all 4 SWDGE queue pairs busy)
    rows_lo = pa * n_cores

    # per-layer: (Dout, table stride in fp16 slots == gather elem size)
    DOUT = [DIM_HID, DIM_HID, DIM_OUT]
    STRIDE = [384, 384, 256]
    GBW = 384  # gb ring chunk width (fp16 slots); layers view it per-stride

    nc = bacc.Bacc("TRN2", target_bir_lowering=False, debug=debug,
                   num_devices=n_cores, num_swdge_queues=4)

    ht_in = nc.dram_tensor("ht", [DIM_IN, nb * BLOCK], dt.float16,
                           kind="ExternalInput")
    w_in = [
        nc.dram_tensor(f"w{l}", [(DIM_IN, DIM_HID, DIM_HID)[l], DOUT[l] + 2],
                       dt.float16, kind="ExternalInput")
        for l in range(3)
    ]
    ixlo_in = nc.dram_tensor("idxlo", [P, meta["i16lo"]], dt.int16,
                             kind="ExternalInput")
    ixhi_in = nc.dram_tensor("idxhi", [P, meta["i16hi"]], dt.int16,
                             kind="ExternalInput")
    seg_in = nc.dram_tensor("seg", [P, meta["seg_cols"]], dt.bfloat16,
                            kind="ExternalInput")
    iota_in = nc.dram_tensor("iota", [P, BLOCK], dt.bfloat16, kind="ExternalInput")
    id16_in = nc.dram_tensor("id16", [P, P], dt.float16, kind="ExternalInput")
    id32_in = nc.dram_tensor("id32", [P, P], dt.float32, kind="ExternalInput")
    out_t = nc.dram_tensor("out", [npc, DIM_OUT], dt.float32,
                           kind="ExternalOutput")

    agi = [nc.dram_tensor(f"agi{l}", [npc, STRIDE[l]], dt.float16)
           for l in range(3)]
    ago = [
        nc.dram_tensor(f"ago{l}", [npc * n_cores + (2 if l == 2 else 0),
                                   STRIDE[l]],
                       dt.float16, addr_space="Shared")
        for l in range(3)
    ]

    def bc_mid(ap2, n):
        return AP(ap2.tensor, ap2.offset, [ap2.ap[0], [0, n], ap2.ap[1]])

    def bc_last(ap2, n):
        return AP(ap2.tensor, ap2.offset, [ap2.ap[0], ap2.ap[1], [0, n]])

    with tile.TileContext(nc) as tc:
        import contextlib

        ctx = contextlib.ExitStack()
        with ctx:
            pers = ctx.enter_context(tc.tile_pool(name="pers", bufs=1))
            pe0 = ctx.enter_context(tc.tile_pool(name="pe0", bufs=2))
            pmsk = ctx.enter_context(tc.tile_pool(name="pmsk", bufs=2))
            psm = ctx.enter_context(tc.tile_pool(name="psm", bufs=2))
            pz = ctx.enter_context(tc.tile_pool(name="pz", bufs=2))
            psum_u = ctx.enter_context(tc.tile_pool(name="psu", bufs=2, space="PSUM"))
            psum_z = ctx.enter_context(tc.tile_pool(name="psz", bufs=2, space="PSUM"))
            psum_tr = ctx.enter_context(tc.tile_pool(name="pstr", bufs=1, space="PSUM"))
            psum_tt = ctx.enter_context(tc.tile_pool(name="pstt", bufs=2, space="PSUM"))
            psum_tb = ctx.enter_context(tc.tile_pool(name="pstb", bufs=1, space="PSUM"))

            nc.gpsimd.load_library(mlp)

            # ---- persistent state ----
            seg_sb = pers.tile([P, meta["seg_cols"]], dt.bfloat16, tag="seg", name="seg_sb")
            ixlo_sb = pers.tile([P, meta["i16lo"]], dt.int16, tag="ixlo", name="ixlo_sb")
            ixhi_sb = pers.tile([P, meta["i16hi"]], dt.int16, tag="ixhi", name="ixhi_sb")
            W_sb = [pers.tile([P, 2, DOUT[l] + 2], dt.float16, tag=f"w{l}", name=f"wsb{l}")
                    for l in range(3)]
            iota_sb = pers.tile([P, BLOCK], dt.bfloat16, tag="iota", name="iota_sb")
            id16 = pers.tile([P, P], dt.float16, tag="id16", name="id16")
            id32 = pers.tile([P, P], dt.float32, tag="id32", name="id32")
            trowt = [pers.tile([1, nb * BLOCK], dt.float16, tag=f"tr{i}", name=f"trowt{i}")
                     for i in range(2)]
            gb = [pers.tile([P, CKMAX, GBW], dt.float16, tag=f"g{i}", name=f"gb{i}")
                  for i in range(RB)]
            ones1 = pers.tile([1, P], dt.float16, tag="ones1", name="ones1")
            # double asm buffer: tail 258:384 stays zero so table rows never
            # carry uninitialized DRAM (layer-2's 256-wide views can read the
            # stale tails of layer-0/1 rows via non-gathered slots)
            asmb = [pers.tile([P, 384], dt.float16, tag=f"asm{i}", name=f"asmb{i}")
                    for i in range(4)]

            nc.sync.dma_start(out=seg_sb[:], in_=seg_in[:, :])
            nc.sync.dma_start(out=ixlo_sb[:], in_=ixlo_in[:, :])
            nc.sync.dma_start(out=ixhi_sb[:], in_=ixhi_in[:, :])
            for l in range(3):
                nc.sync.dma_start(
                    out=W_sb[l][:],
                    in_=w_in[l].ap().rearrange("(k p) d -> p k d", p=P),
                )
            nc.sync.dma_start(out=iota_sb[:], in_=iota_in[:, :])
            nc.sync.dma_start(out=id16[:], in_=id16_in[:, :])
            nc.sync.dma_start(out=id32[:], in_=id32_in[:, :])
            for i in range(RB):
                nc.vector.memset(gb[i][:], 0.0)
            nc.vector.memset(ones1[:], 1.0)
            for i in range(4):
                nc.vector.memset(asmb[i][:, 258:384], 0.0)

            def zphase(b, lhsTs, lnext, tci):
                """lhsTs: kch APs of [P, 128] fp16 transposed activations.
                Computes z_aug for next-layer table, writes agi[lnext],
                stashes t column. Table row: [z(Dn) | 1 | s_fp16]."""
                node0 = b * BLOCK
                bn = min(BLOCK, npc - node0)
                Dn = DOUT[lnext]
                zp = psum_z.tile([P, 258], dt.float32, tag="zp", name="zp")
                kch = len(lhsTs)
                for k in range(kch):
                    nc.tensor.matmul(
                        out=zp[:, : Dn + 2],
                        lhsT=lhsTs[k],
                        rhs=W_sb[lnext][:, k, : Dn + 2],
                        start=(k == 0),
                        stop=(k == kch - 1),
                    )
                asm = asmb[b % 4]
                nc.scalar.activation(asm[:bn, 0:Dn], zp[:bn, 0:Dn], Act.Copy)
                nc.vector.memset(asm[:bn, Dn: Dn + 1], 1.0)
                nc.scalar.activation(
                    asm[:bn, Dn + 1: Dn + 2], zp[:bn, Dn: Dn + 1], Act.Copy
                )
                # t column -> row b*128.. of t_rowT (PE transpose + copy)
                tcl = psm.tile([P, 1], dt.float32, tag="tcl", name="tcl")
                nc.vector.tensor_copy(out=tcl[:], in_=zp[:, Dn + 1: Dn + 2])
                tp = psum_tt.tile([1, P], dt.float32)
                nc.tensor.transpose(
                    out=tp[0:1, :P], in_=tcl[:, :], identity=id32[:, :],
                )
                nc.scalar.activation(
                    trowt[tci][0:1, node0: node0 + P], tp[0:1, :], Act.Copy
                )
                # layers 0/1: write the full 384-slot row (tail zeros) so the
                # table never exposes uninitialized DRAM; layer 2 rows are
                # 256-wide and only 0:130 is ever read row-aligned.
                wcols = Dn + 2 if lnext == 2 else STRIDE[lnext]
                nc.sync.dma_start(
                    out=agi[lnext][node0: node0 + bn, 0:wcols],
                    in_=asm[:bn, 0:wcols],
                )

            pb1 = PB1_BLOCKS * BLOCK
            parts = [(i * 1024, (i + 1) * 1024, n_cores * i * 1024)
                     for i in range(4)]
            parts += [(pa, pa + pb1, n_cores * pa),
                      (pa + pb1, npc, n_cores * (pa + pb1))]
            AGPTS = {7: 0, 15: 1, 23: 2, 31: 3, 43: 4}

            def allgather(l, pi):
                # one full part: output region is exactly contiguous
                S = STRIDE[l]
                a = ago[l].ap()
                r0, r1, base_row = parts[pi]
                rows = r1 - r0
                out_ap = AP(a.tensor, base_row * S,
                            [[rows * S, n_cores], [S, rows], [1, S]])
                nc.gpsimd.collective_compute(
                    "AllGather",
                    bass.mybir.AluOpType.bypass,
                    replica_groups=[list(range(n_cores))],
                    ins=[agi[l].ap()[r0:r1, :].opt()],
                    outs=[out_ap.opt()],
                )

            # ---- layer-1 priming: hT -> z_aug table + t ----
            for b in range(nb):
                node0 = b * BLOCK
                hst = pz.tile([P, 2, P], dt.float16, tag="hst", name="hst")
                nc.sync.dma_start(
                    out=hst[:],
                    in_=ht_in.ap()[:, node0: node0 + P].rearrange(
                        "(k p) d -> p k d", p=P),
                )
                zphase(
                    b,
                    [hst[:, k, :] for k in range(2)],
                    0, 0,
                )
                if b in AGPTS:
                    allgather(0, AGPTS[b])
            allgather(0, 5)

            # ---- 3 edge layers ----
            for l in range(3):
                Dn = DOUT[l]
                DU = Dn + 1       # z cols + ones col
                SSL = Dn + 1      # s slot within the row
                S = STRIDE[l]     # table stride == gather elem width
                last = l == 2
                ag_ap = ago[l].ap()
                lo_tab = AP(ag_ap.tensor, 0, [[S, rows_lo], [1, S]])
                hi_tab = AP(ag_ap.tensor, rows_lo * S,
                            [[S, N - rows_lo], [1, S]])

                for b in range(nb):
                    bl = blocks[b]
                    node0 = b * BLOCK
                    bn = min(BLOCK, npc - node0)
                    cklo, cktot = bl["cklo"], bl["cktot"]
                    g = gb[b % RB][:, :, :]
                    gt, goff, gpap = g.tensor, g.offset, g.ap[0]

                    def gview(c0, nck):
                        return AP(gt, goff + c0 * S,
                                  [gpap, [S, nck], [1, S]])

                    for tab, n, i16off, c0 in [
                        (lo_tab, bl["nlo"], bl["ilo"], 0),
                        (hi_tab, bl["nhi"], bl["ihi"], cklo),
                    ]:
                        ixtab = ixlo_sb if c0 == 0 else ixhi_sb
                        done = 0
                        while done < n:
                            sub = min(n - done, MAXCK * 128)
                            nck = _cdiv(sub, 128)
                            o16 = i16off + done // 16
                            cs = c0 + done // 128
                            nc.gpsimd.dma_gather(
                                gview(cs, nck), tab,
                                ixtab[:, o16: o16 + _cdiv(sub, 16)],
                                sub, sub, S, elem_step=S,
                                queue_num=b % 4,
                            )
                            done += sub
                    # tb[p, j] = t[dst j of block b] via PE outer product
                    tbp = psum_tb.tile([P, BLOCK], dt.float32, tag="tbp", name="tbp")
                    nc.tensor.matmul(
                        out=tbp[:, :],
                        lhsT=ones1[0:1, :],
                        rhs=trowt[l % 2][0:1, node0: node0 + BLOCK],
                        start=True, stop=True,
                    )
                    tb = psm.tile([P, BLOCK], dt.float16, tag="tb16", name="tb16")
                    nc.scalar.activation(tb[:, :], tbp[:, :], Act.Copy)
                    seg_v = seg_sb[:, bl["sgo"]: bl["sgo"] + cktot]
                    pt0 = pmsk.tile([P, CKMAX, BLOCK], dt.bfloat16, tag="pt0", name="pt0")
                    nc.vector.tensor_tensor(
                        out=pt0[:, 0:cktot, :],
                        in0=bc_last(seg_v, BLOCK),
                        in1=bc_mid(iota_sb[:], cktot),
                        op=Alu.is_equal,
                    )
                    sv = AP(gt, goff + SSL, [gpap, [S, cktot]])
                    e0 = pe0.tile([P, CKMAX, BLOCK], dt.float16, tag="e0", name="e0")
                    nc.vector.tensor_tensor(
                        out=e0[:, 0:cktot, :],
                        in0=bc_last(sv, BLOCK),
                        in1=bc_mid(tb[:], cktot),
                        op=Alu.add,
                    )
                    eb = pmsk.tile([P, CKMAX, BLOCK], dt.bfloat16, tag="eb", name="eb")
                    nc.scalar.activation(
                        eb[:, 0:cktot, :], e0[:, 0:cktot, :], Act.Exp
                    )
                    ptx = pmsk.tile([P, CKMAX, BLOCK], dt.bfloat16, tag="ptx", name="ptx")
                    nc.scalar.activation(
                        ptx[:, 0:cktot, :], e0[:, 0:cktot, :], Act.Exp,
                        scale=NEG_SLOPE,
                    )
                    nc.vector.tensor_tensor(
                        out=eb[:, 0:cktot, :],
                        in0=eb[:, 0:cktot, :],
                        in1=ptx[:, 0:cktot, :],
                        op=Alu.max,
                    )
                    nc.vector.tensor_tensor(
                        out=ptx[:, 0:cktot, :],
                        in0=pt0[:, 0:cktot, :],
                        in1=eb[:, 0:cktot, :],
                        op=Alu.mult,
                    )
                    U = psum_u.tile([P, 258], dt.float32, tag="U", name="U")
                    for k in range(cktot):
                        nc.tensor.matmul(
                            out=U[:, :DU],
                            lhsT=ptx[:, k, :],
                            rhs=AP(gt, goff + k * S, [gpap, [1, DU]]),
                            start=(k == 0),
                            stop=(k == cktot - 1),
                        )
                    den = psm.tile([P, 1], dt.float32, tag="den", name="den")
                    nc.vector.tensor_scalar(
                        out=den[:], in0=U[:, Dn: Dn + 1], scalar1=1e-9,
                        scalar2=None, op0=Alu.max,
                    )
                    rec = psm.tile([P, 1], dt.float32, tag="rec", name="rec")
                    nc.vector.reciprocal(rec[:], den[:])
                    xo = psm.tile([P, 256], dt.float32, tag="xo", name="xo")
                    nc.vector.tensor_scalar(
                        out=xo[:, 0:Dn], in0=U[:, 0:Dn], scalar1=rec[:],
                        scalar2=None, op0=Alu.mult,
                    )
                    if last:
                        nc.sync.dma_start(
                            out=out_t[node0: node0 + bn, :],
                            in_=xo[:bn, 0:DIM_OUT],
                        )
                        continue
                    a16 = psm.tile([P, 256], dt.float16, tag="a16", name="a16")
                    if l == 0:
                        nc.scalar.activation(a16[:, 0:Dn], xo[:, 0:Dn], Act.Tanh)
                    else:  # elu
                        mn = psm.tile([P, 256], dt.float32, tag="mn", name="mn")
                        # min(x,0) = -relu(-x); exp via ACT scale=-1 chains
                        nc.scalar.activation(
                            mn[:, 0:Dn], xo[:, 0:Dn], Act.Relu, scale=-1.0,
                        )
                        nc.scalar.activation(
                            mn[:, 0:Dn], mn[:, 0:Dn], Act.Exp, scale=-1.0,
                        )
                        nc.vector.scalar_tensor_tensor(
                            out=mn[:, 0:Dn], in0=xo[:, 0:Dn], scalar=0.0,
                            in1=mn[:, 0:Dn], op0=Alu.max, op1=Alu.add,
                        )
                        nc.scalar.activation(
                            a16[:, 0:Dn], mn[:, 0:Dn], Act.Copy, bias=-1.0,
                        )
                    stage = pz.tile([P, 256], dt.float16, tag="stg", name="stg")
                    for k in range(2):
                        ps = psum_tr.tile([P, P], dt.float16)
                        nc.tensor.transpose(
                            out=ps[:P, :P],
                            in_=a16[:, k * P: (k + 1) * P],
                            identity=id16[:, :],
                        )
                        nc.scalar.activation(
                            stage[:, k * P: (k + 1) * P], ps[:, :], Act.Copy
                        )
                    zphase(b, [stage[:, k * P: (k + 1) * P] for k in range(2)],
                           l + 1, (l + 1) % 2)
                    if b in AGPTS:
                        allgather(l + 1, AGPTS[b])
                if not last:
                    allgather(l + 1, 5)

    nc.compile()
    return nc


# ---------------------------------------------------------------------------
# entry point
# ---------------------------------------------------------------------------

_CACHE = {}


def _prepare(src, dst, n_nodes):
    key = (int(n_nodes), src.tobytes(), dst.tobytes())
    kh = hash(key)
    if kh not in _CACHE:
        meta, per_core = build_plan(src, dst, n_nodes, N_CORES)
        nc = build_nc(meta)
        _CACHE[kh] = (meta, per_core, nc)
    return _CACHE[kh]


def make_in_maps(meta, per_core, h, W1, A1, W2, A2, W3, A3):
    npc = meta["npc"]
    nb = meta["nb"]
    w = [build_waug(W1, A1), build_waug(W2, A2), build_waug(W3, A3)]
    h = np.asarray(h, dtype=np.float32)[meta["perm"]]
    in_maps = []
    for c in range(N_CORES):
        hc = h[c * npc: (c + 1) * npc].astype(np.float16)
        ht = np.zeros((DIM_IN, nb * BLOCK), np.float16)
        ht[:, :npc] = hc.T
        in_maps.append(
            {
                "ht": np.ascontiguousarray(ht),
                "w0": w[0],
                "w1": w[1],
                "w2": w[2],
                "idxlo": per_core["idxlo"][c],
                "idxhi": per_core["idxhi"][c],
                "seg": per_core["seg"][c],
                **const_inputs(),
            }
        )
    return in_maps


def kernel(h, src, dst, n_nodes, W1, A1, W2, A2, W3, A3):
    from concourse.bass_utils import run_bass_kernel_spmd

    n_nodes = int(n_nodes)
    assert n_nodes == N_NODES
    meta, per_core, nc = _prepare(np.asarray(src), np.asarray(dst), n_nodes)

    in_maps = make_in_maps(meta, per_core, h, W1, A1, W2, A2, W3, A3)
    res = run_bass_kernel_spmd(nc, in_maps, core_ids=list(range(N_CORES)))
    npc = meta["npc"]
    pos = np.concatenate([res.results[c]["out"] for c in range(N_CORES)], axis=0)
    out = np.empty((n_nodes, DIM_OUT), np.float32)
    out[meta["perm"]] = pos[:n_nodes]
    return out
